# revision 1
# baseline (speedup 1.0000x reference)
"""Sparse (class-gated bilinear) attention kernel for TRN2, 8 NeuronCores.

Problem shapes (hardcoded): b=2, h=8, s=512, d=64, C=8 classes, B=4 bases.

Math (per b,h):
  W1e[c] = (sum_B softmax(alpha1)[c,B,h] * W1[B,h]) / sqrt(d)   (host)
  W2e[c] = sum_B softmax(alpha2)[c,B,h] * W2[B,h]               (host)
  UT_c   = W1e[c]^T-contraction:  UT_c[n,i] = sum_m W1e[c][m,n] * Q[i,m]
  ST_c   = ST_c[j,i] = sum_n K[j,n] * UT_c[n,i]                 (PE, fp32r)
  scoresT[j,i] = ST_{bmat[i,j]}[j,i] + rpb[i,j]                 (DVE select)
  E = exp(scoresT)           (no max-subtraction needed; |scores| < ~40)
  t_c[j,D] = sum_d V[j,d] W2e[c][d,D]                           (PE)
  outT[D,i] = sum_c sum_j t_c[j,D] * (E . mask_c)[j,i]          (PE, bf16)
  Z[i] = sum_j E[j,i]                                           (PE ones-row)
  out[i,D] = outT[D,i] / Z[i]                                   (host)

Sharding: 16 (b,h) pairs over 8 cores; core k handles b=k//4,
heads (2*(k%4), 2*(k%4)+1). b_mat shared by both heads of a core.
"""

import os
import sys

import numpy as np

if "/opt/trn_rl_repo" not in sys.path:
    sys.path.insert(0, "/opt/trn_rl_repo")

import ml_dtypes

B_, H_, S_, D_, C_ = 2, 8, 512, 64, 8
NCORES = 8
JT = S_ // 128  # 4 j-tiles

# Selection chain dtype is fp32 (reads ST PSUM directly); split/output
# matmul side runs in ELEM (bf16 — contributes only ~0.2% rel err).
ELEM = "bfloat16"

_CACHE = {}


def _softmax(a, axis):
    e = np.exp(a - a.max(axis=axis, keepdims=True))
    return e / e.sum(axis=axis, keepdims=True)


def _build_nc():
    import concourse.bass as bass  # noqa: F401
    import concourse.mybir as mybir
    from concourse import bacc
    from concourse.tile import TileContext

    f32 = mybir.dt.float32
    f32r = mybir.dt.float32r
    f16 = mybir.dt.float16
    ebt = mybir.dt.bfloat16 if ELEM == "bfloat16" else mybir.dt.float32

    nc = bacc.Bacc("TRN2", target_bir_lowering=False, debug=False)

    qt_d = nc.dram_tensor("qt", [2, 64, 512], f32r, kind="ExternalInput").ap()
    kt_d = nc.dram_tensor("kt", [2, 64, 512], f32r, kind="ExternalInput").ap()
    vt_d = nc.dram_tensor("vt", [2, 64, 512], f32r, kind="ExternalInput").ap()
    w1_d = nc.dram_tensor("w1", [2, 64, 512], f32r, kind="ExternalInput").ap()
    w2_d = nc.dram_tensor("w2", [2, 64, 512], f32r, kind="ExternalInput").ap()
    erp_d = nc.dram_tensor("erp", [2, 512, 512], ebt, kind="ExternalInput").ap()
    bmt_d = nc.dram_tensor("bmt", [512, 512], ebt, kind="ExternalInput").ap()
    ot_d = nc.dram_tensor("ot", [2, 65, 512], f32, kind="ExternalOutput").ap()

    EXP = mybir.ActivationFunctionType.Exp
    EQ = mybir.AluOpType.is_equal

    with TileContext(nc) as tc:
        with (
            tc.tile_pool(name="const", bufs=1) as cpool,
            tc.tile_pool(name="inp", bufs=1) as ipool,
            tc.tile_pool(name="mask", bufs=1) as mpool,
            tc.tile_pool(name="work", bufs=5) as wpool,
            tc.tile_pool(name="ec", bufs=24) as epool,
            tc.tile_pool(name="pst", bufs=6, space="PSUM") as pst,
            tc.tile_pool(name="pacc", bufs=1, space="PSUM") as pacc,
        ):
            ones = cpool.tile([128, 1], ebt, tag="ones")
            nc.vector.memset(ones, 1.0)

            # per-class uint16 masks from b_mat^T, shared by both heads on
            # the core; used by copy_predicated (chain) AND split muls
            imasks = [[None] * C_ for _ in range(JT)]
            mpairs = [[None] * (C_ // 2) for _ in range(JT)]
            for jt in range(JT):
                bt = ipool.tile([128, 512], ebt, tag=f"bmt{jt}")
                nc.sync.dma_start(out=bt, in_=bmt_d[jt * 128 : (jt + 1) * 128, :])
                for q in range(C_ // 2):
                    mp = mpool.tile(
                        [128, 1024], mybir.dt.uint16, tag=f"i{jt}_{q}",
                        name=f"i{jt}_{q}",
                    )
                    for h_ in range(2):
                        c = 2 * q + h_
                        im = mp[:, h_ * 512 : (h_ + 1) * 512]
                        nc.vector.tensor_scalar(im, bt, float(c), None, EQ)
                        imasks[jt][c] = im
                    mpairs[jt][q] = mp

            qt, kt, vt, w1, w2 = {}, {}, {}, {}, {}
            ut, tsb = {}, {}
            ot_ps, z_ps = {}, {}
            for p in range(2):
                qt[p] = ipool.tile([64, 512], f32r, tag=f"qt{p}", name=f"qt{p}")
                nc.sync.dma_start(out=qt[p], in_=qt_d[p])
                kt[p] = ipool.tile([64, 512], f32r, tag=f"kt{p}", name=f"kt{p}")
                nc.sync.dma_start(out=kt[p], in_=kt_d[p])
                vt[p] = ipool.tile([64, 512], f32r, tag=f"vt{p}", name=f"vt{p}")
                nc.sync.dma_start(out=vt[p], in_=vt_d[p])
                w1[p] = ipool.tile([64, 512], f32r, tag=f"w1{p}", name=f"w1{p}")
                nc.sync.dma_start(out=w1[p], in_=w1_d[p])
                w2[p] = ipool.tile([64, 512], f32r, tag=f"w2{p}", name=f"w2{p}")
                nc.sync.dma_start(out=w2[p], in_=w2_d[p])

                # UT_c = W1e[c].T-contract @ Q^T : [64, 512] each
                ut[p] = []
                for c in range(C_):
                    up = pst.tile([128, 512], mybir.dt.float32, tag="st")
                    nc.tensor.matmul(
                        up[:64], w1[p][:, c * 64 : (c + 1) * 64], qt[p],
                        start=True, stop=True,
                    )
                    us = ipool.tile([64, 512], f32r, tag=f"ut{p}_{c}")
                    nc.any.tensor_copy(out=us, in_=up[:64])
                    ut[p].append(us)

                # t_all[j-tile] = V-tile @ W2cat : [128, (c,D)=512]
                tsb[p] = []
                for jt in range(JT):
                    tp = pst.tile([128, 512], mybir.dt.float32, tag="st")
                    nc.tensor.matmul(
                        tp, vt[p][:, jt * 128 : (jt + 1) * 128], w2[p],
                        start=True, stop=True,
                    )
                    ts = ipool.tile([128, 512], ebt, tag=f"t{p}_{jt}")
                    nc.any.tensor_copy(out=ts, in_=tp)
                    tsb[p].append(ts)

                ot_ps[p] = pacc.tile([65, 512], mybir.dt.float32, tag=f"o{p}", name=f"ot{p}")
                z_ps[p] = ot_ps[p][64:65]

            # Interleaved (jt, p) steps. Output/Z matmuls for step s are
            # emitted during step s+1 so they never block the next step's
            # ST matmuls in the in-order PE stream.
            pending = None

            def flush_pending():
                et_, ecs_, p_, jt_ = pending
                nc.tensor.matmul(
                    z_ps[p_], ones, et_,
                    start=(jt_ == 0), stop=(jt_ == JT - 1),
                    skip_group_check=True,
                )
                for c in range(C_):
                    nc.tensor.matmul(
                        ot_ps[p_][:64], tsb[p_][jt_][:, c * 64 : (c + 1) * 64],
                        ecs_[c // 2][:, (c % 2) * 512 : (c % 2 + 1) * 512],
                        start=(jt_ == 0 and c == 0),
                        stop=(jt_ == JT - 1 and c == C_ - 1),
                        skip_group_check=True,
                    )

            for jt in range(JT):
                for p in range(2):
                    rp = wpool.tile([128, 512], ebt, tag="rpb")
                    nc.sync.dma_start(
                        out=rp, in_=erp_d[p, jt * 128 : (jt + 1) * 128, :]
                    )
                    # ST matmuls -> PSUM; fp32 selection chain reads the
                    # PSUM banks directly. ACT does the class-0 seed copy.
                    sc = wpool.tile([128, 512], f32, tag="sc")
                    for c in range(C_):
                        sp = pst.tile([128, 512], mybir.dt.float32, tag="st")
                        nc.tensor.matmul(
                            sp, kt[p][:, jt * 128 : (jt + 1) * 128], ut[p][c],
                            start=True, stop=True,
                        )
                        if c == 0:
                            nc.scalar.copy(sc, sp)
                        else:
                            nc.vector.copy_predicated(sc, imasks[jt][c], sp)

                    eraw = wpool.tile([128, 512], ebt, tag="eraw")
                    nc.scalar.activation(eraw, sc, EXP)
                    et = wpool.tile([128, 512], ebt, tag="et")
                    nc.vector.tensor_mul(et, eraw, rp)

                    etb = et[:, None, :].to_broadcast([128, 2, 512])
                    ecs = []
                    for q in range(C_ // 2):
                        ec2 = epool.tile(
                            [128, 1024], ebt, tag="ec", name=f"ec{q}"
                        )
                        eng = nc.gpsimd if q in (1, 3) else nc.vector
                        eng.tensor_mul(
                            ec2.rearrange("p (two f) -> p two f", two=2),
                            etb,
                            mpairs[jt][q].rearrange(
                                "p (two f) -> p two f", two=2
                            ),
                        )
                        ecs.append(ec2)

                    if pending is not None:
                        flush_pending()
                    pending = (et, ecs, p, jt)
            flush_pending()

            for p in range(2):
                os_ = wpool.tile([65, 512], mybir.dt.float32, tag="os")
                nc.scalar.copy(os_, ot_ps[p])
                nc.sync.dma_start(out=ot_d[p], in_=os_)

    nc.compile()
    return nc


def _get_nc():
    if "nc" not in _CACHE:
        _CACHE["nc"] = _build_nc()
    return _CACHE["nc"]


def kernel(**inputs):
    q = np.asarray(inputs["query"], np.float32)
    k = np.asarray(inputs["key"], np.float32)
    v = np.asarray(inputs["value"], np.float32)
    bm = np.asarray(inputs["b_mat"])
    rpb = np.asarray(inputs["rpb"], np.float32)
    W1 = np.asarray(inputs["W1"], np.float32)
    a1 = np.asarray(inputs["alpha1"], np.float32)
    W2 = np.asarray(inputs["W2"], np.float32)
    a2 = np.asarray(inputs["alpha2"], np.float32)
    mask = np.asarray(inputs["mask"])

    W1e = np.einsum("Bhmn,CBh->Chmn", W1, _softmax(a1, 1)) / np.sqrt(D_)
    W2e = np.einsum("BhdD,CBh->ChdD", W2, _softmax(a2, 1))

    eb = ml_dtypes.bfloat16 if ELEM == "bfloat16" else np.float32
    # additive -inf pair mask would go here; spec guarantees mask == ones
    assert mask.all(), "kernel assumes all-ones mask (spec fill=ones)"

    in_maps = []
    for cid in range(NCORES):
        b = cid // 4
        hs = [2 * (cid % 4), 2 * (cid % 4) + 1]
        qt = np.stack([q[b, h].T for h in hs]).astype(np.float32)
        kt = np.stack([k[b, h].T for h in hs]).astype(np.float32)
        vt = np.stack([v[b, h].T for h in hs]).astype(np.float32)
        # [m, C, n] -> [64, 512] per head
        w1 = np.stack(
            [W1e[:, h].transpose(1, 0, 2).reshape(64, 512) for h in hs]
        ).astype(np.float32)
        w2 = np.stack(
            [W2e[:, h].transpose(1, 0, 2).reshape(64, 512) for h in hs]
        ).astype(np.float32)
        erp = np.exp(np.stack([rpb[b, h].T for h in hs])).astype(
            ml_dtypes.bfloat16
        )
        bmt = bm[b].T.astype(np.float32).astype(eb)
        in_maps.append(
            {"qt": qt, "kt": kt, "vt": vt, "w1": w1, "w2": w2,
             "erp": erp, "bmt": bmt}
        )

    import time

    from concourse.bass_utils import run_bass_kernel_spmd

    try:
        res = run_bass_kernel_spmd(
            _get_nc(), in_maps, core_ids=list(range(NCORES))
        )
    except Exception:
        # transient NRT_EXEC_UNIT_UNRECOVERABLE from a previously wedged
        # device clears on redispatch
        time.sleep(5)
        res = run_bass_kernel_spmd(
            _get_nc(), in_maps, core_ids=list(range(NCORES))
        )
    _CACHE["last_res"] = res
    outs = res.results

    out = np.zeros((B_, H_, S_, D_), np.float32)
    for cid in range(NCORES):
        b = cid // 4
        hs = [2 * (cid % 4), 2 * (cid % 4) + 1]
        for p, h in enumerate(hs):
            ot = np.asarray(outs[cid]["ot"][p], np.float32)  # [65, 512]
            out[b, h] = (ot[:64] / ot[64:65]).T
    return out



# revision 9
# speedup vs baseline: 1.0730x; 1.0730x over previous
"""Sparse (class-gated bilinear) attention kernel for TRN2, 8 NeuronCores.

Problem shapes (hardcoded): b=2, h=8, s=512, d=64, C=8 classes, B=4 bases.

Math (per b,h):
  W1e[c] = (sum_B softmax(alpha1)[c,B,h] * W1[B,h]) / sqrt(d)   (host)
  W2e[c] = sum_B softmax(alpha2)[c,B,h] * W2[B,h]               (host)
  UT_c   = W1e[c]^T-contraction:  UT_c[n,i] = sum_m W1e[c][m,n] * Q[i,m]
  ST_c[j,i] = sum_n K[j,n] * UT_c[n,i]                          (PE, fp32r)
  E_c[j,i] = (b_mat[i,j]==c) * exp(ST_c[j,i]) * exp(rpb[i,j])
  t_c[j,D] = sum_d V[j,d] W2e[c][d,D]                           (PE)
  out[D,i] = sum_c sum_j t_c[j,D] * E_c[j,i]                    (PE, bf16)
  Z[i]     = sum_c sum_j E_c[j,i]      (ones column folded into t)
  final[i,D] = out[D,i] / Z[i]                                  (host)

Per-step class split (per [128 j, 512 i] tile):
  chain classes 1..3: ST selected into a persistent sc tile via
      copy_predicated (DVE; sc is memset once, stale lanes stay bounded
      so exp never overflows and masked lanes multiply to 0), ONE exp
      (ACT), then all three masked-E planes via one 4x-mode fused
      scalar_tensor_tensor on host-shifted b_mat (is_equal 0).
  direct classes 0,4..7: exp(ST_c) straight from PSUM (ACT), times
      host-precomputed mask_c*exp(rpb) planes (STT on DVE/Pool; Pool
      never touches PSUM - hardware restriction).

Sharding: 16 (b,h) pairs over 8 cores; core k handles b=k//4,
heads (2*(k%4), 2*(k%4)+1), packed 2-per-tile along partitions.
kt is sent twice (natural + swapped head order) because matmul operands
must share a base partition and class parity selects base 0 or 64.
"""

import sys

import numpy as np

if "/opt/trn_rl_repo" not in sys.path:
    sys.path.insert(0, "/opt/trn_rl_repo")

import ml_dtypes

B_, H_, S_, D_, C_ = 2, 8, 512, 64, 8
NCORES = 8
JT = S_ // 128        # 4 j-tiles
CHAIN = (1, 2, 3)     # selected via predicated chain
DIRECT = (0, 4, 5, 6, 7)  # per-class exp + host mask*exp(rpb)
NCH = len(CHAIN)
NDI = len(DIRECT)

_CACHE = {}


def _softmax(a, axis):
    e = np.exp(a - a.max(axis=axis, keepdims=True))
    return e / e.sum(axis=axis, keepdims=True)


def _build_nc():
    import concourse.bass as bass  # noqa: F401
    import concourse.mybir as mybir
    from concourse import bacc
    from concourse.tile import TileContext

    f32 = mybir.dt.float32
    f32r = mybir.dt.float32r
    bf16 = mybir.dt.bfloat16
    u16 = mybir.dt.uint16

    EXP = mybir.ActivationFunctionType.Exp
    EQ = mybir.AluOpType.is_equal
    MUL = mybir.AluOpType.mult

    nc = bacc.Bacc("TRN2", target_bir_lowering=False, debug=False)

    qt_d = nc.dram_tensor("qt", [128, 512], f32r, kind="ExternalInput").ap()
    kt_d = nc.dram_tensor("kt", [2, 128, 512], f32r, kind="ExternalInput").ap()
    vt_d = nc.dram_tensor("vt", [128, 512], f32r, kind="ExternalInput").ap()
    w1_d = nc.dram_tensor("w1", [128, 512], f32r, kind="ExternalInput").ap()
    w2_d = nc.dram_tensor("w2", [128, 512], f32r, kind="ExternalInput").ap()
    # exp(rpb) per head: [p][par][jt*512+i]
    erp_d = nc.dram_tensor("erp", [2, 128, JT * 512], bf16, kind="ExternalInput").ap()
    # b_mat - c for chain classes: [jt][par][(ci,i)]
    bmt_d = nc.dram_tensor("bmt", [JT, 128, NCH * 512], bf16, kind="ExternalInput").ap()
    # mask_c * exp(rpb) for direct classes: [p][jt][par][(ci,i)]
    mep_d = nc.dram_tensor("mep", [2, JT, 128, NDI * 512], bf16, kind="ExternalInput").ap()
    ot_d = nc.dram_tensor("ot", [2, 65, 512], f32, kind="ExternalOutput").ap()

    with TileContext(nc) as tc:
        with (
            tc.tile_pool(name="inp", bufs=1) as ipool,
            tc.tile_pool(name="work", bufs=4) as wpool,
            tc.tile_pool(name="ec", bufs=3) as epool,
            tc.tile_pool(name="pst", bufs=3, space="PSUM") as pst,
            tc.tile_pool(name="pacc", bufs=1, space="PSUM") as pacc,
        ):
            # --- input DMAs (ordered roughly by first use) ---
            qt = ipool.tile([128, 512], f32r, tag="qt")
            nc.sync.dma_start(out=qt, in_=qt_d)
            w1 = ipool.tile([128, 512], f32r, tag="w1")
            nc.sync.dma_start(out=w1, in_=w1_d)
            kt = []
            for v_ in range(2):
                ktv = ipool.tile([128, 512], f32r, tag=f"kt{v_}", name=f"kt{v_}")
                nc.sync.dma_start(out=ktv, in_=kt_d[v_])
                kt.append(ktv)
            bmt = []
            for jt in range(JT):
                bt = ipool.tile([128, NCH * 512], bf16, tag=f"bmt{jt}", name=f"bmt{jt}")
                if jt == 0:
                    nc.sync.dma_start(out=bt, in_=bmt_d[jt])
                bmt.append(bt)
            erp = []
            for p in range(2):
                ep = ipool.tile([128, JT * 512], bf16, tag=f"erp{p}", name=f"erp{p}")
                nc.sync.dma_start(out=ep, in_=erp_d[p])
                erp.append(ep)
            mep = [[None] * JT for _ in range(2)]
            for p in range(2):
                mp_ = ipool.tile([128, NDI * 512], bf16, tag=f"mep{p}_0", name=f"mep{p}_0")
                nc.sync.dma_start(out=mp_, in_=mep_d[p, 0])
                mep[p][0] = mp_
            vt = ipool.tile([128, 512], f32r, tag="vt")
            nc.sync.dma_start(out=vt, in_=vt_d)
            w2 = ipool.tile([128, 512], f32r, tag="w2")
            nc.sync.dma_start(out=w2, in_=w2_d)
            for jt in range(1, JT):
                nc.sync.dma_start(out=bmt[jt], in_=bmt_d[jt])
                for p in range(2):
                    mp_ = ipool.tile(
                        [128, NDI * 512], bf16, tag=f"mep{p}_{jt}", name=f"mep{p}_{jt}"
                    )
                    nc.sync.dma_start(out=mp_, in_=mep_d[p, jt])
                    mep[p][jt] = mp_

            # --- uint16 chain masks: one fused op per jt ---
            imask = []
            for jt in range(JT):
                im = ipool.tile([128, NCH * 512], u16, tag=f"im{jt}", name=f"im{jt}")
                nc.vector.tensor_scalar(im, bmt[jt], 0.0, None, EQ)
                imask.append(im)

            # persistent selected-score tile; memset once, chain ops
            # overwrite class lanes each step (stale lanes stay bounded)
            sc = ipool.tile([128, 512], f32, tag="sc")
            nc.vector.memset(sc, 0.0)

            # --- setup matmuls: ut pairs and t (ones column for Z) ---
            # ut[p][g] : [128,1024] f32r; class c lives at rows (c%2)*64,
            # cols ((c//2)%2)*512 of tile g=c//4
            ut = {}
            t520 = {}   # t520[p][jp] : [128, 2*8*65] bf16 (j-tile pair)
            for p in range(2):
                rows = slice(p * 64, (p + 1) * 64)
                ut[p] = []
                for g in range(2):
                    up = pst.tile([128, 1024], mybir.dt.float32, tag="st2")
                    for h_ in range(2):
                        cp = 2 * g + h_
                        nc.tensor.matmul(
                            up[:, h_ * 512 : (h_ + 1) * 512],
                            w1[rows, cp * 128 : (cp + 1) * 128], qt[rows, :],
                            start=True, stop=True,
                        )
                    us = ipool.tile([128, 1024], f32r, tag=f"ut{p}_{g}", name=f"ut{p}_{g}")
                    if g == 0:
                        nc.scalar.copy(us, up)
                    else:
                        nc.vector.tensor_copy(out=us, in_=up)
                    ut[p].append(us)
            for p in range(2):
                rows = slice(p * 64, (p + 1) * 64)
                t520[p] = []
                for jp in range(2):
                    tp = pst.tile([128, 1024], mybir.dt.float32, tag="st2")
                    for h_ in range(2):
                        jt = 2 * jp + h_
                        nc.tensor.matmul(
                            tp[:, h_ * 512 : (h_ + 1) * 512],
                            vt[rows, jt * 128 : (jt + 1) * 128], w2[rows, :],
                            start=True, stop=True,
                        )
                    ts = ipool.tile(
                        [128, 2 * C_ * 65], bf16, tag=f"t{p}_{jp}", name=f"t{p}_{jp}"
                    )
                    tsv = ts.rearrange("q (j c e) -> q j c e", j=2, c=C_)
                    eng = nc.scalar if jp == 0 else nc.vector
                    if eng is nc.scalar:
                        nc.scalar.copy(
                            tsv[:, :, :, 0:64],
                            tp.rearrange("q (j c e) -> q j c e", j=2, c=C_),
                        )
                    else:
                        nc.vector.tensor_copy(
                            out=tsv[:, :, :, 0:64],
                            in_=tp.rearrange("q (j c e) -> q j c e", j=2, c=C_),
                        )
                    nc.vector.memset(tsv[:, :, :, 64:65], 1.0)
                    t520[p].append(ts)

            ot_ps = {}
            for p in range(2):
                ot_ps[p] = pacc.tile([65, 512], mybir.dt.float32, tag=f"o{p}", name=f"op{p}")

            # --- main steps; output matmuls deferred one step ---
            pending = None

            def flush_pending():
                ec_, p_, jt_ = pending
                tsv = t520[p_][jt_ // 2]
                for c in range(C_):
                    off = ((jt_ % 2) * C_ + c) * 65
                    nc.tensor.matmul(
                        ot_ps[p_],
                        tsv[:, off : off + 65],
                        ec_[:, c * 512 : (c + 1) * 512],
                        start=(jt_ == 0 and c == 0),
                        stop=(jt_ == JT - 1 and c == C_ - 1),
                        skip_group_check=True,
                    )

            # ST pair tiles: [0,4],[5,6],[7,1],[2,3] — direct classes land
            # early so ACT starts while chain matmuls still run
            st_pairs = [(0, 4), (5, 6), (7, 1), (2, 3)]

            for jt in range(JT):
                for p in range(2):
                    jcols = slice(jt * 128, (jt + 1) * 128)
                    sp = [None] * C_
                    for pair in st_pairs:
                        s2 = pst.tile([128, 1024], mybir.dt.float32, tag="st2")
                        for h_, c in enumerate(pair):
                            m = (c % 2) * 64
                            ktv = kt[0] if (c % 2) == p else kt[1]
                            g, gh = c // 4, (c // 2) % 2
                            nc.tensor.matmul(
                                s2[:, h_ * 512 : (h_ + 1) * 512],
                                ktv[m : m + 64, jcols],
                                ut[p][g][m : m + 64, gh * 512 : (gh + 1) * 512],
                                start=True, stop=True,
                            )
                            sp[c] = s2[:, h_ * 512 : (h_ + 1) * 512]

                    # direct classes: exp straight off PSUM on ACT
                    exc = {}
                    for c in DIRECT:
                        ex = wpool.tile([128, 512], bf16, tag=f"ex{c}", name=f"ex{c}")
                        nc.scalar.activation(ex, sp[c], EXP)
                        exc[c] = ex
                    # chain: predicated merges into persistent sc (DVE)
                    for ci, c in enumerate(CHAIN):
                        nc.vector.copy_predicated(
                            sc, imask[jt][:, ci * 512 : (ci + 1) * 512], sp[c]
                        )
                    eraw = wpool.tile([128, 512], bf16, tag="eraw")
                    nc.scalar.activation(eraw, sc, EXP)
                    et = wpool.tile([128, 512], bf16, tag="et")
                    nc.vector.tensor_mul(
                        et, eraw, erp[p][:, jt * 512 : (jt + 1) * 512]
                    )

                    ec = epool.tile([128, C_ * 512], bf16, tag="ec")
                    # all chain masked-E planes in one 4x STT
                    etb = et[:, None, :].to_broadcast([128, NCH, 512])
                    nc.vector.scalar_tensor_tensor(
                        ec[:, CHAIN[0] * 512 : (CHAIN[-1] + 1) * 512].rearrange(
                            "q (c f) -> q c f", c=NCH
                        ),
                        bmt[jt].rearrange("q (c f) -> q c f", c=NCH),
                        0.0, etb, EQ, MUL,
                    )
                    # direct masked-E planes: mep_c * exp_c  (4x STT on
                    # DVE; plain tensor_mul on Pool - no STT opcode there)
                    for ci, c in enumerate(DIRECT):
                        mslice = mep[p][jt][:, ci * 512 : (ci + 1) * 512]
                        eslice = ec[:, c * 512 : (c + 1) * 512]
                        if ci >= NDI - 2:
                            nc.gpsimd.tensor_mul(eslice, mslice, exc[c])
                        else:
                            nc.vector.scalar_tensor_tensor(
                                eslice, mslice, 1.0, exc[c], MUL, MUL
                            )

                    if pending is not None:
                        flush_pending()
                    pending = (ec, p, jt)
            flush_pending()

            for p in range(2):
                os_ = wpool.tile([65, 512], mybir.dt.float32, tag="os")
                nc.scalar.copy(os_, ot_ps[p])
                nc.sync.dma_start(out=ot_d[p], in_=os_)

    nc.compile()
    return nc


def _get_nc():
    if "nc" not in _CACHE:
        _CACHE["nc"] = _build_nc()
    return _CACHE["nc"]


def kernel(**inputs):
    q = np.asarray(inputs["query"], np.float32)
    k = np.asarray(inputs["key"], np.float32)
    v = np.asarray(inputs["value"], np.float32)
    bm = np.asarray(inputs["b_mat"])
    rpb = np.asarray(inputs["rpb"], np.float32)
    W1 = np.asarray(inputs["W1"], np.float32)
    a1 = np.asarray(inputs["alpha1"], np.float32)
    W2 = np.asarray(inputs["W2"], np.float32)
    a2 = np.asarray(inputs["alpha2"], np.float32)
    mask = np.asarray(inputs["mask"])

    W1e = np.einsum("Bhmn,CBh->Chmn", W1, _softmax(a1, 1)) / np.sqrt(D_)
    W2e = np.einsum("BhdD,CBh->ChdD", W2, _softmax(a2, 1))

    bf = ml_dtypes.bfloat16
    # additive -inf pair mask would go here; spec guarantees mask == ones
    assert mask.all(), "kernel assumes all-ones mask (spec fill=ones)"

    in_maps = []
    for cid in range(NCORES):
        b = cid // 4
        hs = [2 * (cid % 4), 2 * (cid % 4) + 1]
        qt = np.concatenate([q[b, h].T for h in hs], 0).astype(np.float32)
        kt = np.stack([
            np.concatenate([k[b, h].T for h in hh], 0)
            for hh in (hs, hs[::-1])
        ]).astype(np.float32)
        vt = np.concatenate([v[b, h].T for h in hs], 0).astype(np.float32)
        w1 = np.concatenate(
            [W1e[:, h].transpose(1, 0, 2).reshape(64, 512) for h in hs], 0
        ).astype(np.float32)
        w2 = np.concatenate(
            [W2e[:, h].transpose(1, 0, 2).reshape(64, 512) for h in hs], 0
        ).astype(np.float32)
        # erp[p, par, jt*512+i] = exp(rpb[b, h_p, i, jt*128+par])
        erp_h = [np.exp(rpb[b, h]).T for h in hs]  # [j, i]
        erp = np.stack(
            [e.reshape(JT, 128, 512).transpose(1, 0, 2).reshape(128, JT * 512)
             for e in erp_h]
        ).astype(bf)
        bmt_t = bm[b].T.astype(np.float32).reshape(JT, 128, 512)  # [jt, par, i]
        bmt = np.stack(
            [np.stack([bmt_t[jt] - c for c in CHAIN], 1)
             .reshape(128, NCH * 512) for jt in range(JT)]
        ).astype(bf)
        # mep[p, jt, par, ci*512+i] = (bmt==DIRECT[ci]) * erp
        mep = np.empty((2, JT, 128, NDI * 512), np.float32)
        for pi, e in enumerate(erp_h):
            e_t = e.reshape(JT, 128, 512)
            for jt in range(JT):
                mep[pi, jt] = np.stack(
                    [(bmt_t[jt] == c) * e_t[jt] for c in DIRECT], 1
                ).reshape(128, NDI * 512)
        mep = mep.astype(bf)
        in_maps.append(
            {"qt": qt, "kt": kt, "vt": vt, "w1": w1, "w2": w2,
             "erp": erp, "bmt": bmt, "mep": mep}
        )

    import time

    from concourse.bass_utils import run_bass_kernel_spmd

    try:
        res = run_bass_kernel_spmd(
            _get_nc(), in_maps, core_ids=list(range(NCORES))
        )
    except Exception:
        # transient NRT_EXEC_UNIT_UNRECOVERABLE from a previously wedged
        # device clears on redispatch
        time.sleep(5)
        res = run_bass_kernel_spmd(
            _get_nc(), in_maps, core_ids=list(range(NCORES))
        )
    _CACHE["last_res"] = res
    outs = res.results

    out = np.zeros((B_, H_, S_, D_), np.float32)
    for cid in range(NCORES):
        b = cid // 4
        hs = [2 * (cid % 4), 2 * (cid % 4) + 1]
        for p, h in enumerate(hs):
            ot = np.asarray(outs[cid]["ot"][p], np.float32)  # [65, 512]
            out[b, h] = (ot[:64] / ot[64:65]).T
    return out


# revision 10
# speedup vs baseline: 1.3237x; 1.2336x over previous
"""Sparse (class-gated bilinear) attention kernel for TRN2, 8 NeuronCores.

Problem shapes (hardcoded): b=2, h=8, s=512, d=64, C=8 classes, B=4 bases.

Math (per b,h):
  W1e[c] = (sum_B softmax(alpha1)[c,B,h] * W1[B,h]) / sqrt(d)   (host)
  W2e[c] = sum_B softmax(alpha2)[c,B,h] * W2[B,h]               (host)
  UT_c   = W1e[c]^T-contraction:  UT_c[n,i] = sum_m W1e[c][m,n] * Q[i,m]
  ST_c[j,i] = sum_n K[j,n] * UT_c[n,i]                          (PE, fp32r)
  mep_c[j,i] = (b_mat[i,j]==c) * exp(rpb[i,j])                  (host)
  E_c[j,i] = mep_c[j,i] * exp(ST_c[j,i])
  t_c[j,D] = sum_d V[j,d] W2e[c][d,D]                           (PE)
  out[D,i] = sum_c sum_j t_c[j,D] * E_c[j,i]                    (PE, bf16)
  Z[i]     = sum_c sum_j E_c[j,i]      (ones column folded into t)
  final[i,D] = out[D,i] / Z[i]                                  (host)

Per-step class split (per [128 j, 512 i] tile):
  chain classes 1..3: ST selected into a persistent sc tile via
      copy_predicated (DVE; masks = mep_c > 0, made on-chip with one 4x
      tensor_scalar per j-tile; sc is memset once, stale lanes stay
      bounded so exp never overflows and masked lanes multiply to 0),
      ONE exp (ACT), then the three masked-E planes via one broadcast
      tensor_mul over the contiguous chain slice of mep.
  direct classes 0,4..7: exp(ST_c) straight from PSUM (ACT, paired
      [128,1024] where the ST pair allows), times mep_c (tensor_mul on
      DVE/Pool; Pool never touches PSUM and has no fused-STT opcode).

Sharding: 16 (b,h) pairs over 8 cores; core k handles b=k//4,
heads (2*(k%4), 2*(k%4)+1), packed 2-per-tile along partitions.
kt is sent twice (natural + swapped head order) because matmul operands
must share a base partition and class parity selects base 0 or 64.
"""

import sys

import numpy as np

if "/opt/trn_rl_repo" not in sys.path:
    sys.path.insert(0, "/opt/trn_rl_repo")

import ml_dtypes

B_, H_, S_, D_, C_ = 2, 8, 512, 64, 8
NCORES = 8
JT = S_ // 128            # 4 j-tiles
CHAIN = (1, 2, 3)         # selected via predicated chain
NCH = len(CHAIN)

_CACHE = {}


def _softmax(a, axis):
    e = np.exp(a - a.max(axis=axis, keepdims=True))
    return e / e.sum(axis=axis, keepdims=True)


def _build_nc():
    import concourse.bass as bass  # noqa: F401
    import concourse.mybir as mybir
    from concourse import bacc
    from concourse.tile import TileContext

    f32 = mybir.dt.float32
    f32r = mybir.dt.float32r
    bf16 = mybir.dt.bfloat16
    u16 = mybir.dt.uint16

    EXP = mybir.ActivationFunctionType.Exp
    GT = mybir.AluOpType.is_gt

    nc = bacc.Bacc("TRN2", target_bir_lowering=False, debug=False)

    qt_d = nc.dram_tensor("qt", [128, 512], f32r, kind="ExternalInput").ap()
    kt_d = nc.dram_tensor("kt", [2, 128, 512], f32r, kind="ExternalInput").ap()
    vt_d = nc.dram_tensor("vt", [128, 512], f32r, kind="ExternalInput").ap()
    w1_d = nc.dram_tensor("w1", [128, 512], f32r, kind="ExternalInput").ap()
    w2_d = nc.dram_tensor("w2", [128, 512], f32r, kind="ExternalInput").ap()
    # (b_mat==c) * exp(rpb), all 8 classes: [p][jt][par][(c,i)]
    mep_d = nc.dram_tensor("mep", [2, JT, 128, C_ * 512], bf16, kind="ExternalInput").ap()
    ot_d = nc.dram_tensor("ot", [2, 65, 512], f32, kind="ExternalOutput").ap()

    with TileContext(nc) as tc:
        with (
            tc.tile_pool(name="inp", bufs=1) as ipool,
            tc.tile_pool(name="work", bufs=4) as wpool,
            tc.tile_pool(name="ec", bufs=3) as epool,
            tc.tile_pool(name="pst", bufs=3, space="PSUM") as pst,
            tc.tile_pool(name="pacc", bufs=1, space="PSUM") as pacc,
        ):
            # --- input DMAs (ordered roughly by first use) ---
            mep = [[None] * JT for _ in range(2)]

            def mep_dma(p, jt):
                mp_ = ipool.tile(
                    [128, C_ * 512], bf16, tag=f"mep{p}_{jt}", name=f"mep{p}_{jt}"
                )
                nc.sync.dma_start(out=mp_, in_=mep_d[p, jt])
                mep[p][jt] = mp_

            mep_dma(0, 0)
            qt = ipool.tile([128, 512], f32r, tag="qt")
            nc.sync.dma_start(out=qt, in_=qt_d)
            w1 = ipool.tile([128, 512], f32r, tag="w1")
            nc.sync.dma_start(out=w1, in_=w1_d)
            kt = []
            for v_ in range(2):
                ktv = ipool.tile([128, 512], f32r, tag=f"kt{v_}", name=f"kt{v_}")
                nc.sync.dma_start(out=ktv, in_=kt_d[v_])
                kt.append(ktv)
            mep_dma(1, 0)
            vt = ipool.tile([128, 512], f32r, tag="vt")
            nc.sync.dma_start(out=vt, in_=vt_d)
            w2 = ipool.tile([128, 512], f32r, tag="w2")
            nc.sync.dma_start(out=w2, in_=w2_d)
            for jt in range(1, JT):
                for p in range(2):
                    mep_dma(p, jt)

            # persistent selected-score tile; memset once, chain ops
            # overwrite class lanes each step (stale lanes stay bounded)
            sc = ipool.tile([128, 512], f32, tag="sc")
            nc.vector.memset(sc, 0.0)

            # --- setup matmuls: ut pairs and t (ones column for Z) ---
            # ut[p][g] : [128,1024] f32r; class c lives at rows (c%2)*64,
            # cols ((c//2)%2)*512 of tile g=c//4
            ut = {}
            t520 = {}   # t520[p][jp] : [128, 2*8*65] bf16 (j-tile pair)
            for p in range(2):
                rows = slice(p * 64, (p + 1) * 64)
                ut[p] = []
                for g in range(2):
                    up = pst.tile([128, 1024], mybir.dt.float32, tag="st2")
                    for h_ in range(2):
                        cp = 2 * g + h_
                        nc.tensor.matmul(
                            up[:, h_ * 512 : (h_ + 1) * 512],
                            w1[rows, cp * 128 : (cp + 1) * 128], qt[rows, :],
                            start=True, stop=True,
                        )
                    us = ipool.tile([128, 1024], f32r, tag=f"ut{p}_{g}", name=f"ut{p}_{g}")
                    if g == 0:
                        nc.scalar.copy(us, up)
                    else:
                        nc.vector.tensor_copy(out=us, in_=up)
                    ut[p].append(us)
            for p in range(2):
                rows = slice(p * 64, (p + 1) * 64)
                t520[p] = []
                for jp in range(2):
                    tp = pst.tile([128, 1024], mybir.dt.float32, tag="st2")
                    for h_ in range(2):
                        jt = 2 * jp + h_
                        nc.tensor.matmul(
                            tp[:, h_ * 512 : (h_ + 1) * 512],
                            vt[rows, jt * 128 : (jt + 1) * 128], w2[rows, :],
                            start=True, stop=True,
                        )
                    ts = ipool.tile(
                        [128, 2 * C_ * 65], bf16, tag=f"t{p}_{jp}", name=f"t{p}_{jp}"
                    )
                    tsv = ts.rearrange("q (j c e) -> q j c e", j=2, c=C_)
                    if jp == 0:
                        nc.scalar.copy(
                            tsv[:, :, :, 0:64],
                            tp.rearrange("q (j c e) -> q j c e", j=2, c=C_),
                        )
                    else:
                        nc.vector.tensor_copy(
                            out=tsv[:, :, :, 0:64],
                            in_=tp.rearrange("q (j c e) -> q j c e", j=2, c=C_),
                        )
                    nc.gpsimd.memset(tsv[:, :, :, 64:65], 1.0)
                    t520[p].append(ts)

            ot_ps = {}
            for p in range(2):
                ot_ps[p] = pacc.tile([65, 512], mybir.dt.float32, tag=f"o{p}", name=f"op{p}")

            # --- main steps; output matmuls deferred one step ---
            pending = None

            def flush_pending():
                ec_, p_, jt_ = pending
                tsv = t520[p_][jt_ // 2]
                for c in range(C_):
                    off = ((jt_ % 2) * C_ + c) * 65
                    nc.tensor.matmul(
                        ot_ps[p_],
                        tsv[:, off : off + 65],
                        ec_[:, c * 512 : (c + 1) * 512],
                        start=(jt_ == 0 and c == 0),
                        stop=(jt_ == JT - 1 and c == C_ - 1),
                        skip_group_check=True,
                    )

            # ST pair tiles: direct pairs first so ACT starts early; the
            # class-1 half of the (7,1) pair only feeds the pred chain
            st_pairs = [(0, 4), (5, 6), (7, 1), (2, 3)]

            imask = [None] * JT
            for jt in range(JT):
                for p in range(2):
                    if p == 0:
                        # chain masks: mep_c > 0 (erp is exp() so > 0)
                        im = ipool.tile(
                            [128, NCH * 512], u16, tag=f"im{jt}", name=f"im{jt}"
                        )
                        nc.vector.tensor_scalar(
                            im,
                            mep[0][jt][:, CHAIN[0] * 512 : (CHAIN[-1] + 1) * 512],
                            0.0, None, GT,
                        )
                        imask[jt] = im

                    jcols = slice(jt * 128, (jt + 1) * 128)
                    sp = [None] * C_
                    spair = {}
                    for pair in st_pairs:
                        s2 = pst.tile([128, 1024], mybir.dt.float32, tag="st2")
                        spair[pair] = s2
                        for h_, c in enumerate(pair):
                            m = (c % 2) * 64
                            ktv = kt[0] if (c % 2) == p else kt[1]
                            g, gh = c // 4, (c // 2) % 2
                            nc.tensor.matmul(
                                s2[:, h_ * 512 : (h_ + 1) * 512],
                                ktv[m : m + 64, jcols],
                                ut[p][g][m : m + 64, gh * 512 : (gh + 1) * 512],
                                start=True, stop=True,
                            )
                            sp[c] = s2[:, h_ * 512 : (h_ + 1) * 512]

                    # direct exps off PSUM on ACT: two full pairs + one half
                    ex04 = wpool.tile([128, 1024], bf16, tag="ex04")
                    nc.scalar.activation(ex04, spair[(0, 4)], EXP)
                    ex56 = wpool.tile([128, 1024], bf16, tag="ex56")
                    nc.scalar.activation(ex56, spair[(5, 6)], EXP)
                    ex7 = wpool.tile([128, 512], bf16, tag="ex7")
                    nc.scalar.activation(ex7, sp[7], EXP)
                    # chain: predicated merges into persistent sc (DVE)
                    for ci, c in enumerate(CHAIN):
                        nc.vector.copy_predicated(
                            sc, imask[jt][:, ci * 512 : (ci + 1) * 512], sp[c]
                        )
                    eraw = wpool.tile([128, 512], bf16, tag="eraw")
                    nc.scalar.activation(eraw, sc, EXP)

                    mj = mep[p][jt]
                    ec = epool.tile([128, C_ * 512], bf16, tag="ec")
                    # chain masked-E planes: one broadcast tensor_mul
                    erb = eraw[:, None, :].to_broadcast([128, NCH, 512])
                    nc.vector.tensor_mul(
                        ec[:, CHAIN[0] * 512 : (CHAIN[-1] + 1) * 512].rearrange(
                            "q (c f) -> q c f", c=NCH
                        ),
                        mj[:, CHAIN[0] * 512 : (CHAIN[-1] + 1) * 512].rearrange(
                            "q (c f) -> q c f", c=NCH
                        ),
                        erb,
                    )
                    # direct masked-E planes: mep_c * exp_c
                    nc.vector.tensor_mul(
                        ec[:, 0:512], mj[:, 0:512], ex04[:, 0:512]
                    )
                    nc.vector.tensor_mul(
                        ec[:, 4 * 512 : 5 * 512], mj[:, 4 * 512 : 5 * 512],
                        ex04[:, 512:1024],
                    )
                    nc.gpsimd.tensor_mul(
                        ec[:, 5 * 512 : 6 * 512], mj[:, 5 * 512 : 6 * 512],
                        ex56[:, 0:512],
                    )
                    nc.gpsimd.tensor_mul(
                        ec[:, 6 * 512 : 7 * 512], mj[:, 6 * 512 : 7 * 512],
                        ex56[:, 512:1024],
                    )
                    nc.gpsimd.tensor_mul(
                        ec[:, 7 * 512 : 8 * 512], mj[:, 7 * 512 : 8 * 512], ex7
                    )

                    if pending is not None:
                        flush_pending()
                    pending = (ec, p, jt)
            flush_pending()

            for p in range(2):
                os_ = wpool.tile([65, 512], mybir.dt.float32, tag="os")
                nc.scalar.copy(os_, ot_ps[p])
                nc.sync.dma_start(out=ot_d[p], in_=os_)

    nc.compile()
    return nc


def _get_nc():
    if "nc" not in _CACHE:
        _CACHE["nc"] = _build_nc()
    return _CACHE["nc"]


def kernel(**inputs):
    q = np.asarray(inputs["query"], np.float32)
    k = np.asarray(inputs["key"], np.float32)
    v = np.asarray(inputs["value"], np.float32)
    bm = np.asarray(inputs["b_mat"])
    rpb = np.asarray(inputs["rpb"], np.float32)
    W1 = np.asarray(inputs["W1"], np.float32)
    a1 = np.asarray(inputs["alpha1"], np.float32)
    W2 = np.asarray(inputs["W2"], np.float32)
    a2 = np.asarray(inputs["alpha2"], np.float32)
    mask = np.asarray(inputs["mask"])

    W1e = np.einsum("Bhmn,CBh->Chmn", W1, _softmax(a1, 1)) / np.sqrt(D_)
    W2e = np.einsum("BhdD,CBh->ChdD", W2, _softmax(a2, 1))

    bf = ml_dtypes.bfloat16
    # additive -inf pair mask would go here; spec guarantees mask == ones
    assert mask.all(), "kernel assumes all-ones mask (spec fill=ones)"

    in_maps = []
    for cid in range(NCORES):
        b = cid // 4
        hs = [2 * (cid % 4), 2 * (cid % 4) + 1]
        qt = np.concatenate([q[b, h].T for h in hs], 0).astype(np.float32)
        kt = np.stack([
            np.concatenate([k[b, h].T for h in hh], 0)
            for hh in (hs, hs[::-1])
        ]).astype(np.float32)
        vt = np.concatenate([v[b, h].T for h in hs], 0).astype(np.float32)
        w1 = np.concatenate(
            [W1e[:, h].transpose(1, 0, 2).reshape(64, 512) for h in hs], 0
        ).astype(np.float32)
        w2 = np.concatenate(
            [W2e[:, h].transpose(1, 0, 2).reshape(64, 512) for h in hs], 0
        ).astype(np.float32)
        # mep[p, jt, par, c*512+i] = (bmt[jt,par,i]==c) * exp(rpb)[j,i]
        bmt_t = bm[b].T.astype(np.int32).reshape(JT, 128, 512)  # [jt, par, i]
        mep = np.empty((2, JT, 128, C_ * 512), np.float32)
        for pi, h in enumerate(hs):
            e_t = np.exp(rpb[b, h]).T.reshape(JT, 128, 512)
            for jt in range(JT):
                mep[pi, jt] = np.concatenate(
                    [(bmt_t[jt] == c) * e_t[jt] for c in range(C_)], 1
                )
        mep = mep.astype(bf)
        in_maps.append(
            {"qt": qt, "kt": kt, "vt": vt, "w1": w1, "w2": w2, "mep": mep}
        )

    import time

    from concourse.bass_utils import run_bass_kernel_spmd

    try:
        res = run_bass_kernel_spmd(
            _get_nc(), in_maps, core_ids=list(range(NCORES))
        )
    except Exception:
        # transient NRT_EXEC_UNIT_UNRECOVERABLE from a previously wedged
        # device clears on redispatch
        time.sleep(5)
        res = run_bass_kernel_spmd(
            _get_nc(), in_maps, core_ids=list(range(NCORES))
        )
    _CACHE["last_res"] = res
    outs = res.results

    out = np.zeros((B_, H_, S_, D_), np.float32)
    for cid in range(NCORES):
        b = cid // 4
        hs = [2 * (cid % 4), 2 * (cid % 4) + 1]
        for p, h in enumerate(hs):
            ot = np.asarray(outs[cid]["ot"][p], np.float32)  # [65, 512]
            out[b, h] = (ot[:64] / ot[64:65]).T
    return out


# revision 11
# speedup vs baseline: 1.3302x; 1.0049x over previous
"""Sparse (class-gated bilinear) attention kernel for TRN2, 8 NeuronCores.

Problem shapes (hardcoded): b=2, h=8, s=512, d=64, C=8 classes, B=4 bases.

Math (per b,h):
  W1e[c] = (sum_B softmax(alpha1)[c,B,h] * W1[B,h]) / sqrt(d)   (host)
  W2e[c] = sum_B softmax(alpha2)[c,B,h] * W2[B,h]               (host)
  UT_c   = W1e[c]^T-contraction:  UT_c[n,i] = sum_m W1e[c][m,n] * Q[i,m]
  ST_c[j,i] = sum_n K[j,n] * UT_c[n,i]                          (PE, fp32r)
  mep_c[j,i] = (b_mat[i,j]==c) * exp(rpb[i,j])                  (host)
  E_c[j,i] = mep_c[j,i] * exp(ST_c[j,i])
  t_c[j,D] = sum_d V[j,d] W2e[c][d,D]                           (PE)
  out[D,i] = sum_c sum_j t_c[j,D] * E_c[j,i]                    (PE, bf16)
  Z[i]     = sum_c sum_j E_c[j,i]      (ones column folded into t)
  final[i,D] = out[D,i] / Z[i]                                  (host)

Per-step class split (per [128 j, 512 i] tile):
  chain classes 1,2,3: ST selected into an alternating pair of sc
      tiles via copy_predicated (DVE; masks = mep_c > 0, one 4x
      tensor_scalar per j-tile; sc is memset once, stale lanes stay
      bounded so exp never overflows and masked lanes multiply to 0),
      ONE exp (ACT), then the three masked-E planes via one broadcast
      tensor_mul over the contiguous chain slice of mep.
  direct classes 0,4,5,6,7: exp(ST_c) straight from PSUM (ACT, paired
      [128,1024]), times mep_c (tensor_mul on DVE/Pool; Pool never
      touches PSUM and has no fused-STT opcode).

Class storage order in mep/ec tiles is [0,4,5,6,7,1,2,3] so the two
DVE direct muls and two of the Pool muls each fuse into one
[128,1024] op and the chain slice stays contiguous.

Sharding: 16 (b,h) pairs over 8 cores; core k handles b=k//4,
heads (2*(k%4), 2*(k%4)+1), packed 2-per-tile along partitions.
kt is sent twice (natural + swapped head order) because matmul operands
must share a base partition and class parity selects base 0 or 64.
"""

import sys

import numpy as np

if "/opt/trn_rl_repo" not in sys.path:
    sys.path.insert(0, "/opt/trn_rl_repo")

import ml_dtypes

B_, H_, S_, D_, C_ = 2, 8, 512, 64, 8
NCORES = 8
JT = S_ // 128            # 4 j-tiles
CORDER = (0, 4, 5, 6, 7, 1, 2, 3)   # class -> slice position
CPOS = {c: i for i, c in enumerate(CORDER)}
CHAIN = (1, 2, 3)
NCH = len(CHAIN)

_CACHE = {}


def _softmax(a, axis):
    e = np.exp(a - a.max(axis=axis, keepdims=True))
    return e / e.sum(axis=axis, keepdims=True)


def _build_nc():
    import concourse.bass as bass  # noqa: F401
    import concourse.mybir as mybir
    from concourse import bacc
    from concourse.tile import TileContext

    f32 = mybir.dt.float32
    f32r = mybir.dt.float32r
    bf16 = mybir.dt.bfloat16
    u16 = mybir.dt.uint16

    EXP = mybir.ActivationFunctionType.Exp
    GT = mybir.AluOpType.is_gt

    nc = bacc.Bacc("TRN2", target_bir_lowering=False, debug=False)

    qt_d = nc.dram_tensor("qt", [128, 512], f32r, kind="ExternalInput").ap()
    kt_d = nc.dram_tensor("kt", [2, 128, 512], f32r, kind="ExternalInput").ap()
    vt_d = nc.dram_tensor("vt", [128, 512], f32r, kind="ExternalInput").ap()
    w1_d = nc.dram_tensor("w1", [128, 512], f32r, kind="ExternalInput").ap()
    w2_d = nc.dram_tensor("w2", [128, 512], f32r, kind="ExternalInput").ap()
    # (b_mat==c) * exp(rpb) in CORDER: [p][jt][par][(ci,i)]
    mep_d = nc.dram_tensor("mep", [2, JT, 128, C_ * 512], bf16, kind="ExternalInput").ap()
    ot_d = nc.dram_tensor("ot", [2, 65, 512], f32, kind="ExternalOutput").ap()

    with TileContext(nc) as tc:
        with (
            tc.tile_pool(name="inp", bufs=1) as ipool,
            tc.tile_pool(name="work", bufs=4) as wpool,
            tc.tile_pool(name="ec", bufs=3) as epool,
            tc.tile_pool(name="pst", bufs=3, space="PSUM") as pst,
            tc.tile_pool(name="pacc", bufs=1, space="PSUM") as pacc,
        ):
            # --- input DMAs, ordered by first use ---
            qt = ipool.tile([128, 512], f32r, tag="qt")
            nc.sync.dma_start(out=qt, in_=qt_d)
            w1 = ipool.tile([128, 512], f32r, tag="w1")
            nc.sync.dma_start(out=w1, in_=w1_d)
            kt = []
            for v_ in range(2):
                ktv = ipool.tile([128, 512], f32r, tag=f"kt{v_}", name=f"kt{v_}")
                nc.sync.dma_start(out=ktv, in_=kt_d[v_])
                kt.append(ktv)
            mep = [[None] * JT for _ in range(2)]

            def mep_dma(p, jt):
                mp_ = ipool.tile(
                    [128, C_ * 512], bf16, tag=f"mep{p}_{jt}", name=f"mep{p}_{jt}"
                )
                nc.sync.dma_start(out=mp_, in_=mep_d[p, jt])
                mep[p][jt] = mp_

            mep_dma(0, 0)
            vt = ipool.tile([128, 512], f32r, tag="vt")
            nc.sync.dma_start(out=vt, in_=vt_d)
            w2 = ipool.tile([128, 512], f32r, tag="w2")
            nc.sync.dma_start(out=w2, in_=w2_d)
            mep_dma(1, 0)
            for jt in range(1, JT):
                for p in range(2):
                    mep_dma(p, jt)

            # two alternating selected-score tiles; memset once, chain
            # ops overwrite class lanes (stale lanes stay bounded)
            sc = []
            for s_ in range(2):
                st = ipool.tile([128, 512], f32, tag=f"sc{s_}", name=f"sc{s_}")
                nc.vector.memset(st, 0.0)
                sc.append(st)

            # --- setup matmuls: ut pairs and t (ones column for Z) ---
            # ut[p][g] : [128,1024] f32r; class c at rows (c%2)*64,
            # cols ((c//2)%2)*512 of tile g=c//4
            ut = {}
            t520 = {}   # t520[p][jp] : [128, 2*8*65] bf16 (j-tile pair)
            for p in range(2):
                rows = slice(p * 64, (p + 1) * 64)
                ut[p] = []
                for g in range(2):
                    up = pst.tile([128, 1024], mybir.dt.float32, tag="st2")
                    for h_ in range(2):
                        cp = 2 * g + h_
                        nc.tensor.matmul(
                            up[:, h_ * 512 : (h_ + 1) * 512],
                            w1[rows, cp * 128 : (cp + 1) * 128], qt[rows, :],
                            start=True, stop=True,
                        )
                    us = ipool.tile([128, 1024], f32r, tag=f"ut{p}_{g}", name=f"ut{p}_{g}")
                    nc.scalar.copy(us, up)
                    ut[p].append(us)
            for p in range(2):
                rows = slice(p * 64, (p + 1) * 64)
                t520[p] = []
                for jp in range(2):
                    tp = pst.tile([128, 1024], mybir.dt.float32, tag="st2")
                    for h_ in range(2):
                        jt = 2 * jp + h_
                        nc.tensor.matmul(
                            tp[:, h_ * 512 : (h_ + 1) * 512],
                            vt[rows, jt * 128 : (jt + 1) * 128], w2[rows, :],
                            start=True, stop=True,
                        )
                    ts = ipool.tile(
                        [128, 2 * C_ * 65], bf16, tag=f"t{p}_{jp}", name=f"t{p}_{jp}"
                    )
                    tsv = ts.rearrange("q (j c e) -> q j c e", j=2, c=C_)
                    nc.scalar.copy(
                        tsv[:, :, :, 0:64],
                        tp.rearrange("q (j c e) -> q j c e", j=2, c=C_),
                    )
                    nc.gpsimd.memset(tsv[:, :, :, 64:65], 1.0)
                    t520[p].append(ts)

            ot_ps = {}
            for p in range(2):
                ot_ps[p] = pacc.tile([65, 512], mybir.dt.float32, tag=f"o{p}", name=f"op{p}")

            # --- main steps; output matmuls deferred one step ---
            pending = None

            def flush_pending():
                ec_, p_, jt_ = pending
                tsv = t520[p_][jt_ // 2]
                for c in range(C_):
                    off = ((jt_ % 2) * C_ + c) * 65
                    nc.tensor.matmul(
                        ot_ps[p_],
                        tsv[:, off : off + 65],
                        ec_[:, CPOS[c] * 512 : (CPOS[c] + 1) * 512],
                        start=(jt_ == 0 and c == 0),
                        stop=(jt_ == JT - 1 and c == C_ - 1),
                        skip_group_check=True,
                    )

            # ST pair tiles: direct pairs first so ACT starts early; the
            # class-1 half of the (7,1) pair only feeds the pred chain
            st_pairs = [(0, 4), (5, 6), (7, 1), (2, 3)]

            imask = [None] * JT
            step = 0
            for jt in range(JT):
                for p in range(2):
                    if p == 0:
                        # chain masks: mep_c > 0 (erp is exp() so > 0)
                        im = ipool.tile(
                            [128, NCH * 512], u16, tag=f"im{jt}", name=f"im{jt}"
                        )
                        nc.vector.tensor_scalar(
                            im, mep[0][jt][:, 5 * 512 :], 0.0, None, GT
                        )
                        imask[jt] = im

                    jcols = slice(jt * 128, (jt + 1) * 128)
                    sp = [None] * C_
                    spair = {}
                    for pair in st_pairs:
                        s2 = pst.tile([128, 1024], mybir.dt.float32, tag="st2")
                        spair[pair] = s2
                        for h_, c in enumerate(pair):
                            m = (c % 2) * 64
                            ktv = kt[0] if (c % 2) == p else kt[1]
                            g, gh = c // 4, (c // 2) % 2
                            nc.tensor.matmul(
                                s2[:, h_ * 512 : (h_ + 1) * 512],
                                ktv[m : m + 64, jcols],
                                ut[p][g][m : m + 64, gh * 512 : (gh + 1) * 512],
                                start=True, stop=True,
                            )
                            sp[c] = s2[:, h_ * 512 : (h_ + 1) * 512]

                    # direct exps off PSUM on ACT: two full pairs + a half
                    ex04 = wpool.tile([128, 1024], bf16, tag="ex04")
                    nc.scalar.activation(ex04, spair[(0, 4)], EXP)
                    ex56 = wpool.tile([128, 1024], bf16, tag="ex56")
                    nc.scalar.activation(ex56, spair[(5, 6)], EXP)
                    ex7 = wpool.tile([128, 512], bf16, tag="ex7")
                    nc.scalar.activation(ex7, sp[7], EXP)
                    # chain: predicated merges into alternating sc (DVE)
                    scs = sc[step % 2]
                    for ci, c in enumerate(CHAIN):
                        nc.vector.copy_predicated(
                            scs, imask[jt][:, ci * 512 : (ci + 1) * 512], sp[c]
                        )
                    eraw = wpool.tile([128, 512], bf16, tag="eraw")
                    nc.scalar.activation(eraw, scs, EXP)

                    mj = mep[p][jt]
                    ec = epool.tile([128, C_ * 512], bf16, tag="ec")
                    # chain masked-E planes: one broadcast tensor_mul
                    erb = eraw[:, None, :].to_broadcast([128, NCH, 512])
                    nc.vector.tensor_mul(
                        ec[:, 5 * 512 :].rearrange("q (c f) -> q c f", c=NCH),
                        mj[:, 5 * 512 :].rearrange("q (c f) -> q c f", c=NCH),
                        erb,
                    )
                    # direct masked-E planes: mep_c * exp_c (paired)
                    nc.vector.tensor_mul(
                        ec[:, 0:1024], mj[:, 0:1024], ex04
                    )
                    nc.gpsimd.tensor_mul(
                        ec[:, 1024:2048], mj[:, 1024:2048], ex56
                    )
                    nc.gpsimd.tensor_mul(
                        ec[:, 2048:2560], mj[:, 2048:2560], ex7
                    )

                    if pending is not None:
                        flush_pending()
                    pending = (ec, p, jt)
                    step += 1
            flush_pending()

            for p in range(2):
                os_ = wpool.tile([65, 512], mybir.dt.float32, tag="os")
                nc.scalar.copy(os_, ot_ps[p])
                nc.sync.dma_start(out=ot_d[p], in_=os_)

    nc.compile()
    return nc


def _get_nc():
    if "nc" not in _CACHE:
        _CACHE["nc"] = _build_nc()
    return _CACHE["nc"]


def kernel(**inputs):
    q = np.asarray(inputs["query"], np.float32)
    k = np.asarray(inputs["key"], np.float32)
    v = np.asarray(inputs["value"], np.float32)
    bm = np.asarray(inputs["b_mat"])
    rpb = np.asarray(inputs["rpb"], np.float32)
    W1 = np.asarray(inputs["W1"], np.float32)
    a1 = np.asarray(inputs["alpha1"], np.float32)
    W2 = np.asarray(inputs["W2"], np.float32)
    a2 = np.asarray(inputs["alpha2"], np.float32)
    mask = np.asarray(inputs["mask"])

    W1e = np.einsum("Bhmn,CBh->Chmn", W1, _softmax(a1, 1)) / np.sqrt(D_)
    W2e = np.einsum("BhdD,CBh->ChdD", W2, _softmax(a2, 1))

    bf = ml_dtypes.bfloat16
    # additive -inf pair mask would go here; spec guarantees mask == ones
    assert mask.all(), "kernel assumes all-ones mask (spec fill=ones)"

    in_maps = []
    for cid in range(NCORES):
        b = cid // 4
        hs = [2 * (cid % 4), 2 * (cid % 4) + 1]
        qt = np.concatenate([q[b, h].T for h in hs], 0).astype(np.float32)
        kt = np.stack([
            np.concatenate([k[b, h].T for h in hh], 0)
            for hh in (hs, hs[::-1])
        ]).astype(np.float32)
        vt = np.concatenate([v[b, h].T for h in hs], 0).astype(np.float32)
        w1 = np.concatenate(
            [W1e[:, h].transpose(1, 0, 2).reshape(64, 512) for h in hs], 0
        ).astype(np.float32)
        w2 = np.concatenate(
            [W2e[:, h].transpose(1, 0, 2).reshape(64, 512) for h in hs], 0
        ).astype(np.float32)
        # mep[p, jt, par, ci*512+i] = (bmt[jt,par,i]==CORDER[ci]) * exp(rpb)[j,i]
        bmt_t = bm[b].T.astype(np.int32).reshape(JT, 128, 512)  # [jt, par, i]
        mep = np.empty((2, JT, 128, C_ * 512), np.float32)
        for pi, h in enumerate(hs):
            e_t = np.exp(rpb[b, h]).T.reshape(JT, 128, 512)
            for jt in range(JT):
                mep[pi, jt] = np.concatenate(
                    [(bmt_t[jt] == c) * e_t[jt] for c in CORDER], 1
                )
        mep = mep.astype(bf)
        in_maps.append(
            {"qt": qt, "kt": kt, "vt": vt, "w1": w1, "w2": w2, "mep": mep}
        )

    import time

    from concourse.bass_utils import run_bass_kernel_spmd

    try:
        res = run_bass_kernel_spmd(
            _get_nc(), in_maps, core_ids=list(range(NCORES))
        )
    except Exception:
        # transient NRT_EXEC_UNIT_UNRECOVERABLE from a previously wedged
        # device clears on redispatch
        time.sleep(5)
        res = run_bass_kernel_spmd(
            _get_nc(), in_maps, core_ids=list(range(NCORES))
        )
    _CACHE["last_res"] = res
    outs = res.results

    out = np.zeros((B_, H_, S_, D_), np.float32)
    for cid in range(NCORES):
        b = cid // 4
        hs = [2 * (cid % 4), 2 * (cid % 4) + 1]
        for p, h in enumerate(hs):
            ot = np.asarray(outs[cid]["ot"][p], np.float32)  # [65, 512]
            out[b, h] = (ot[:64] / ot[64:65]).T
    return out


# revision 17
# speedup vs baseline: 1.5063x; 1.1324x over previous
"""Sparse (class-gated bilinear) attention kernel for TRN2, 8 NeuronCores.

Problem shapes (hardcoded): b=2, h=8, s=512, d=64, C=8 classes, B=4 bases.

Math (per b,h):
  W1e[c] = (sum_B softmax(alpha1)[c,B,h] * W1[B,h]) / sqrt(d)   (host)
  W2e[c] = sum_B softmax(alpha2)[c,B,h] * W2[B,h]               (host)
  UT_c   = W1e[c]^T-contraction:  UT_c[n,i] = sum_m W1e[c][m,n] * Q[i,m]
  ST_c[j,i] = sum_n K[j,n] * UT_c[n,i]                          (PE, fp32r)
  mep_c[j,i] = (b_mat[i,j]==c) * exp(rpb[i,j])                  (host)
  E_c[j,i] = mep_c[j,i] * exp(ST_c[j,i])
  t_c[j,D] = sum_d V[j,d] W2e[c][d,D]                           (PE)
  out[D,i] = sum_c sum_j t_c[j,D] * E_c[j,i]                    (PE, bf16)
  Z[i]     = sum_c sum_j E_c[j,i]      (ones column folded into t)
  final[i,D] = out[D,i] / Z[i]                                  (host)

Per-step class split (per [128 j, 512 i] tile):
  chain classes 1,2,3: ST selected into an alternating pair of sc
      tiles via copy_predicated (DVE; masks = mep_c > 0, one 4x
      tensor_scalar per j-tile; sc is memset once, stale lanes stay
      bounded so exp never overflows and masked lanes multiply to 0),
      ONE exp (ACT), then the three masked-E planes via one broadcast
      tensor_mul over the contiguous chain slice of mep.
  direct classes 0,4,5,6,7: exp(ST_c) straight from PSUM (ACT, paired
      [128,1024]), times mep_c (tensor_mul on DVE/Pool; Pool never
      touches PSUM and has no fused-STT opcode).

Class storage order in mep/ec tiles is [0,4,5,6,7,1,2,3] so the two
DVE direct muls and two of the Pool muls each fuse into one
[128,1024] op and the chain slice stays contiguous.

Sharding: 16 (b,h) pairs over 8 cores; core k handles b=k//4,
heads (2*(k%4), 2*(k%4)+1), packed 2-per-tile along partitions.
kt is sent twice (natural + swapped head order) because matmul operands
must share a base partition and class parity selects base 0 or 64.
"""

import sys

import numpy as np

if "/opt/trn_rl_repo" not in sys.path:
    sys.path.insert(0, "/opt/trn_rl_repo")

import ml_dtypes

B_, H_, S_, D_, C_ = 2, 8, 512, 64, 8
NCORES = 8
JT = S_ // 128            # 4 j-tiles
CORDER = (0, 4, 5, 6, 7, 1, 2, 3)   # class -> slice position
CPOS = {c: i for i, c in enumerate(CORDER)}
CHAIN = (1, 2, 3)
NCH = len(CHAIN)

_CACHE = {}


def _softmax(a, axis):
    e = np.exp(a - a.max(axis=axis, keepdims=True))
    return e / e.sum(axis=axis, keepdims=True)


def _build_nc():
    import concourse.bass as bass  # noqa: F401
    import concourse.mybir as mybir
    from concourse import bacc
    from concourse.tile import TileContext

    f32 = mybir.dt.float32
    f32r = mybir.dt.float32r
    bf16 = mybir.dt.bfloat16
    u16 = mybir.dt.uint16

    EXP = mybir.ActivationFunctionType.Exp
    GT = mybir.AluOpType.is_gt

    nc = bacc.Bacc("TRN2", target_bir_lowering=False, debug=False)

    qt_d = nc.dram_tensor("qt", [128, 512], f32r, kind="ExternalInput").ap()
    kt_d = nc.dram_tensor("kt", [2, 128, 512], f32r, kind="ExternalInput").ap()
    vt_d = nc.dram_tensor("vt", [128, 512], f32r, kind="ExternalInput").ap()
    w1_d = nc.dram_tensor("w1", [128, 512], f32r, kind="ExternalInput").ap()
    w2_d = nc.dram_tensor("w2", [128, 512], f32r, kind="ExternalInput").ap()
    # (b_mat==c) * exp(rpb) in CORDER: [p][jt][par][(ci,i)]
    mep_d = nc.dram_tensor("mep", [2, JT, 128, C_ * 512], bf16, kind="ExternalInput").ap()
    ot_d = nc.dram_tensor("ot", [2, 65, 512], f32, kind="ExternalOutput").ap()

    with TileContext(nc) as tc:
        with (
            tc.tile_pool(name="inp", bufs=1) as ipool,
            tc.tile_pool(name="work", bufs=4) as wpool,
            tc.tile_pool(name="ec", bufs=3) as epool,
            tc.tile_pool(name="pst", bufs=1, space="PSUM") as pst,
            tc.tile_pool(name="pacc", bufs=1, space="PSUM") as pacc,
        ):
            # --- input DMAs, ordered by first use ---
            qt = ipool.tile([128, 512], f32r, tag="qt")
            nc.sync.dma_start(out=qt, in_=qt_d)
            w1 = ipool.tile([128, 512], f32r, tag="w1")
            nc.sync.dma_start(out=w1, in_=w1_d)
            kt = []
            for v_ in range(2):
                ktv = ipool.tile([128, 512], f32r, tag=f"kt{v_}", name=f"kt{v_}")
                nc.sync.dma_start(out=ktv, in_=kt_d[v_])
                kt.append(ktv)
            mep = [[None] * JT for _ in range(2)]

            def mep_dma(p, jt):
                mp_ = ipool.tile(
                    [128, C_ * 512], bf16, tag=f"mep{p}_{jt}", name=f"mep{p}_{jt}"
                )
                nc.sync.dma_start(out=mp_, in_=mep_d[p, jt])
                mep[p][jt] = mp_

            mep_dma(0, 0)
            vt = ipool.tile([128, 512], f32r, tag="vt")
            nc.sync.dma_start(out=vt, in_=vt_d)
            w2 = ipool.tile([128, 512], f32r, tag="w2")
            nc.sync.dma_start(out=w2, in_=w2_d)
            mep_dma(1, 0)
            for jt in range(1, JT):
                for p in range(2):
                    mep_dma(p, jt)

            # two alternating selected-score tiles; memset once, chain
            # ops overwrite class lanes (stale lanes stay bounded)
            sc = []
            for s_ in range(2):
                st = ipool.tile([128, 512], f32, tag=f"sc{s_}", name=f"sc{s_}")
                nc.vector.memset(st, 0.0)
                sc.append(st)

            # --- setup matmuls: ut pairs and t (ones column for Z) ---
            # ut[p][g] : [128,1024] f32r; class c at rows (c%2)*64,
            # cols ((c//2)%2)*512 of tile g=c//4
            ut = {}
            t520 = {}   # t520[p][jp] : [128, 2*8*65] bf16 (j-tile pair)
            setup_tags = ["sda", "s71", "s23", "sda", "s71", "s23", "sda", "s71"]
            sti = iter(setup_tags)
            for p in range(2):
                rows = slice(p * 64, (p + 1) * 64)
                ut[p] = []
                for g in range(2):
                    up = pst.tile([128, 1024], mybir.dt.float32, tag=next(sti))
                    for h_ in range(2):
                        cp = 2 * g + h_
                        nc.tensor.matmul(
                            up[:, h_ * 512 : (h_ + 1) * 512],
                            w1[rows, cp * 128 : (cp + 1) * 128], qt[rows, :],
                            start=True, stop=True,
                        )
                    us = ipool.tile([128, 1024], f32r, tag=f"ut{p}_{g}", name=f"ut{p}_{g}")
                    nc.scalar.copy(us, up)
                    ut[p].append(us)
            for p in range(2):
                rows = slice(p * 64, (p + 1) * 64)
                t520[p] = []
                for jp in range(2):
                    tp = pst.tile([128, 1024], mybir.dt.float32, tag=next(sti))
                    for h_ in range(2):
                        jt = 2 * jp + h_
                        nc.tensor.matmul(
                            tp[:, h_ * 512 : (h_ + 1) * 512],
                            vt[rows, jt * 128 : (jt + 1) * 128], w2[rows, :],
                            start=True, stop=True,
                        )
                    ts = ipool.tile(
                        [128, 2 * C_ * 65], bf16, tag=f"t{p}_{jp}", name=f"t{p}_{jp}"
                    )
                    tsv = ts.rearrange("q (j c e) -> q j c e", j=2, c=C_)
                    nc.scalar.copy(
                        tsv[:, :, :, 0:64],
                        tp.rearrange("q (j c e) -> q j c e", j=2, c=C_),
                    )
                    nc.gpsimd.memset(tsv[:, :, :, 64:65], 1.0)
                    t520[p].append(ts)

            ot_ps = {}
            for p in range(2):
                ot_ps[p] = pacc.tile([65, 512], mybir.dt.float32, tag=f"o{p}", name=f"op{p}")

            # --- main steps; output matmuls deferred one step ---
            pending = None

            def flush_pending():
                ec_, p_, jt_ = pending
                tsv = t520[p_][jt_ // 2]
                for c in range(C_):
                    off = ((jt_ % 2) * C_ + c) * 65
                    nc.tensor.matmul(
                        ot_ps[p_],
                        tsv[:, off : off + 65],
                        ec_[:, CPOS[c] * 512 : (CPOS[c] + 1) * 512],
                        start=(jt_ == 0 and c == 0),
                        stop=(jt_ == JT - 1 and c == C_ - 1),
                        skip_group_check=True,
                    )

            # ST pair tiles with dedicated PSUM tags so buffer recycling
            # follows each pair's own consumer: direct pairs free via
            # early ACT exps, chain pairs via the early preds. The
            # class-1 half of the (7,1) pair only feeds the pred chain.
            st_pairs = [(0, 4), (7, 1), (2, 3), (5, 6)]
            pair_tag = {(0, 4): "sda", (7, 1): "s71", (2, 3): "s23", (5, 6): "sda"}

            imask = [None] * JT
            step = 0
            for jt in range(JT):
                for p in range(2):
                    if p == 0:
                        # chain masks: mep_c > 0 (erp is exp() so > 0)
                        im = ipool.tile(
                            [128, NCH * 512], u16, tag=f"im{jt}", name=f"im{jt}"
                        )
                        nc.vector.tensor_scalar(
                            im, mep[0][jt][:, 5 * 512 :], 0.0, None, GT
                        )
                        imask[jt] = im

                    jcols = slice(jt * 128, (jt + 1) * 128)
                    sp = [None] * C_
                    spair = {}
                    for pair in st_pairs:
                        s2 = pst.tile([128, 1024], mybir.dt.float32, tag=pair_tag[pair])
                        spair[pair] = s2
                        for h_, c in enumerate(pair):
                            m = (c % 2) * 64
                            ktv = kt[0] if (c % 2) == p else kt[1]
                            g, gh = c // 4, (c // 2) % 2
                            nc.tensor.matmul(
                                s2[:, h_ * 512 : (h_ + 1) * 512],
                                ktv[m : m + 64, jcols],
                                ut[p][g][m : m + 64, gh * 512 : (gh + 1) * 512],
                                start=True, stop=True,
                            )
                            sp[c] = s2[:, h_ * 512 : (h_ + 1) * 512]

                    # direct exps off PSUM on ACT: two full pairs + a half
                    ex04 = wpool.tile([128, 1024], bf16, tag="ex04")
                    nc.scalar.activation(ex04, spair[(0, 4)], EXP)
                    ex7 = wpool.tile([128, 512], bf16, tag="ex7")
                    nc.scalar.activation(ex7, sp[7], EXP)
                    ex56 = wpool.tile([128, 1024], bf16, tag="ex56")
                    nc.scalar.activation(ex56, spair[(5, 6)], EXP)
                    # chain: predicated merges into alternating sc (DVE)
                    scs = sc[step % 2]
                    for ci, c in enumerate(CHAIN):
                        nc.vector.copy_predicated(
                            scs, imask[jt][:, ci * 512 : (ci + 1) * 512], sp[c]
                        )
                    eraw = wpool.tile([128, 512], bf16, tag="eraw")
                    nc.scalar.activation(eraw, scs, EXP)

                    mj = mep[p][jt]
                    ec = epool.tile([128, C_ * 512], bf16, tag="ec")
                    # chain masked-E planes: one broadcast tensor_mul
                    erb = eraw[:, None, :].to_broadcast([128, NCH, 512])
                    nc.vector.tensor_mul(
                        ec[:, 5 * 512 :].rearrange("q (c f) -> q c f", c=NCH),
                        mj[:, 5 * 512 :].rearrange("q (c f) -> q c f", c=NCH),
                        erb,
                    )
                    # direct masked-E planes: mep_c * exp_c (paired)
                    nc.vector.tensor_mul(
                        ec[:, 0:1024], mj[:, 0:1024], ex04
                    )
                    nc.gpsimd.tensor_mul(
                        ec[:, 1024:2048], mj[:, 1024:2048], ex56
                    )
                    nc.gpsimd.tensor_mul(
                        ec[:, 2048:2560], mj[:, 2048:2560], ex7
                    )

                    if pending is not None:
                        fp, fjt = pending[1], pending[2]
                        flush_pending()
                        if fjt == JT - 1:
                            # head fp fully accumulated: drain it now so
                            # the output DMA overlaps the remaining steps
                            os_ = wpool.tile(
                                [65, 512], mybir.dt.float32, tag=f"os{fp}",
                                name=f"os{fp}",
                            )
                            nc.scalar.copy(os_, ot_ps[fp])
                            nc.sync.dma_start(out=ot_d[fp], in_=os_)
                    pending = (ec, p, jt)
                    step += 1
            fp = pending[1]
            flush_pending()
            os_ = wpool.tile([65, 512], mybir.dt.float32, tag=f"os{fp}", name=f"os{fp}")
            nc.scalar.copy(os_, ot_ps[fp])
            nc.sync.dma_start(out=ot_d[fp], in_=os_)

    nc.compile()
    return nc


def _get_nc():
    if "nc" not in _CACHE:
        _CACHE["nc"] = _build_nc()
    return _CACHE["nc"]


def kernel(**inputs):
    q = np.asarray(inputs["query"], np.float32)
    k = np.asarray(inputs["key"], np.float32)
    v = np.asarray(inputs["value"], np.float32)
    bm = np.asarray(inputs["b_mat"])
    rpb = np.asarray(inputs["rpb"], np.float32)
    W1 = np.asarray(inputs["W1"], np.float32)
    a1 = np.asarray(inputs["alpha1"], np.float32)
    W2 = np.asarray(inputs["W2"], np.float32)
    a2 = np.asarray(inputs["alpha2"], np.float32)
    mask = np.asarray(inputs["mask"])

    W1e = np.einsum("Bhmn,CBh->Chmn", W1, _softmax(a1, 1)) / np.sqrt(D_)
    W2e = np.einsum("BhdD,CBh->ChdD", W2, _softmax(a2, 1))

    bf = ml_dtypes.bfloat16
    # additive -inf pair mask would go here; spec guarantees mask == ones
    assert mask.all(), "kernel assumes all-ones mask (spec fill=ones)"

    in_maps = []
    for cid in range(NCORES):
        b = cid // 4
        hs = [2 * (cid % 4), 2 * (cid % 4) + 1]
        qt = np.concatenate([q[b, h].T for h in hs], 0).astype(np.float32)
        kt = np.stack([
            np.concatenate([k[b, h].T for h in hh], 0)
            for hh in (hs, hs[::-1])
        ]).astype(np.float32)
        vt = np.concatenate([v[b, h].T for h in hs], 0).astype(np.float32)
        w1 = np.concatenate(
            [W1e[:, h].transpose(1, 0, 2).reshape(64, 512) for h in hs], 0
        ).astype(np.float32)
        w2 = np.concatenate(
            [W2e[:, h].transpose(1, 0, 2).reshape(64, 512) for h in hs], 0
        ).astype(np.float32)
        # mep[p, jt, par, ci*512+i] = (bmt[jt,par,i]==CORDER[ci]) * exp(rpb)[j,i]
        bmt_t = bm[b].T.astype(np.int32).reshape(JT, 128, 512)  # [jt, par, i]
        mep = np.empty((2, JT, 128, C_ * 512), np.float32)
        for pi, h in enumerate(hs):
            e_t = np.exp(rpb[b, h]).T.reshape(JT, 128, 512)
            for jt in range(JT):
                mep[pi, jt] = np.concatenate(
                    [(bmt_t[jt] == c) * e_t[jt] for c in CORDER], 1
                )
        mep = mep.astype(bf)
        in_maps.append(
            {"qt": qt, "kt": kt, "vt": vt, "w1": w1, "w2": w2, "mep": mep}
        )

    import time

    from concourse.bass_utils import run_bass_kernel_spmd

    try:
        res = run_bass_kernel_spmd(
            _get_nc(), in_maps, core_ids=list(range(NCORES))
        )
    except Exception:
        # transient NRT_EXEC_UNIT_UNRECOVERABLE from a previously wedged
        # device clears on redispatch
        time.sleep(5)
        res = run_bass_kernel_spmd(
            _get_nc(), in_maps, core_ids=list(range(NCORES))
        )
    _CACHE["last_res"] = res
    outs = res.results

    out = np.zeros((B_, H_, S_, D_), np.float32)
    for cid in range(NCORES):
        b = cid // 4
        hs = [2 * (cid % 4), 2 * (cid % 4) + 1]
        for p, h in enumerate(hs):
            ot = np.asarray(outs[cid]["ot"][p], np.float32)  # [65, 512]
            out[b, h] = (ot[:64] / ot[64:65]).T
    return out
